# revision 1
# baseline (speedup 1.0000x reference)
"""GCN layer (gather -> segment-mean -> concat -> linear) on 8 TRN2 NeuronCores.

Strategy (dst-sharded, fully replicated feature table):
  - The 50000 output nodes are split across 8 cores (6250 each). Each core
    handles exactly the edges whose dst lands in its range; no cross-core
    communication.
  - Per core, nodes are bin-packed into 49 groups of <=128 so that group
    degree sums are balanced (minimizes the shared padded tile schedule).
  - Messages feature[src] are fetched with the GPSIMD dma_gather custom
    instruction (int16 indices => feature is split into a lo half
    [0, 32768) and a hi half [32768, 50000)).
  - Segment-sum on the TensorEngine: per 128-edge tile,
    psum_hT[D, n] += matmul(lhsT=msgs[e, D], rhs=S[e, n]) where
    S[e, n] = (dstv[e] == n) * w[e], w[e] = 1/max(deg(dst_e), 1).
    S is built for a whole group in two batched DVE ops (is_equal + mult
    with stride-0 broadcast access patterns).
  - Output linear layer: psum_out[n, dout] = xT.T @ W1t + featT.T @ W2t
    + ones.T @ b, three accumulating matmuls per group.
"""

import sys

for _p in ("/opt/trn_rl_repo",):
    if _p not in sys.path:
        sys.path.insert(0, _p)

import numpy as np

import concourse.bass as bass
import concourse.mybir as mybir
from concourse import bacc, library_config
from concourse.bass_utils import run_bass_kernel_spmd
from concourse.tile import TileContext
from concourse.vector_clock import ScopedClock

N_NODES = 50000
N_EDGES = 800000
D = 128
D_OUT = 128
N_CORES = 8
NODES_PER_CORE = N_NODES // N_CORES  # 6250
GROUPS_PER_CORE = (NODES_PER_CORE + 127) // 128  # 49
SLOTS_PER_CORE = GROUPS_PER_CORE * 128  # 6272 (padded)
LO_SPLIT = 32768  # int16-index limit for dma_gather
SENTINEL = 1000.0  # dstv value that matches no iota column
G_CHUNK = 4  # groups per dma_gather call


def _patched_drain_and_barrier(self, tick_clock, wait_clock):
    # The staged walrus build rejects Drain instructions carrying more than
    # one sem wait; split the tail-drain waits onto individual nops.
    probe = self.nc.sync.nop()
    if probe.ins.sync_info is None:
        probe.ins.sync_info = mybir.SyncInfo(on_wait=[], on_update=[])
    wait_clock.add_sem_waits(probe.ins, ScopedClock({None: tick_clock.global_clock}))
    si = probe.ins.sync_info
    waits = list(si.on_wait or [])
    si.on_wait = waits[:1]
    for w in waits[1:]:
        n = self.nc.sync.nop()
        n.ins.sync_info = mybir.SyncInfo(on_wait=[w], on_update=[])
    self.nc.sync.drain()
    self.nc.all_engine_barrier()
    popped = self.nc._tile_sem_poison_stack.pop()
    assert popped is self._sem_poison
    self.nc.clear_and_free_semaphores(list(self.sems.allocated().values()))
    self.nc.all_engine_barrier()


def _apply_tile_patch():
    import concourse.tile as ctile

    ctile.TileContext._drain_and_barrier = _patched_drain_and_barrier


def _wrap_idxs(flat):
    """[N] int16 -> [128, N//16]: position i at [i%16 + 16k, i//16], k=0..7."""
    n = flat.shape[0]
    assert n % 16 == 0
    arr = flat.reshape(n // 16, 16).T
    return np.ascontiguousarray(np.tile(arr, (8, 1)))


def _pack_groups(deg_slice):
    """Greedy balanced bin-packing of 6250 nodes into 49 groups of <=128.

    Returns group_of [6250], slot_of [6250] (slot in [0,128))."""
    n = deg_slice.shape[0]
    order = np.argsort(-deg_slice, kind="stable")
    loads = np.zeros(GROUPS_PER_CORE)
    counts = np.zeros(GROUPS_PER_CORE, np.int64)
    group_of = np.zeros(n, np.int64)
    slot_of = np.zeros(n, np.int64)
    for node in order:
        masked = np.where(counts < 128, loads, np.inf)
        g = int(np.argmin(masked))
        group_of[node] = g
        slot_of[node] = counts[g]
        counts[g] += 1
        loads[g] += deg_slice[node]
    return group_of, slot_of


def _prep_core(src, dst, drecip, deg, core):
    """Host-side partitioning for one core: bin-packed groups + per-group
    per-half edge lists (idx, dstv, wv)."""
    lo_node = core * NODES_PER_CORE
    hi_node = lo_node + NODES_PER_CORE
    deg_slice = deg[lo_node:hi_node]
    group_of, slot_of = _pack_groups(deg_slice)

    sel = (dst >= lo_node) & (dst < hi_node)
    e_src = src[sel]
    e_ldst = dst[sel] - lo_node
    grp = group_of[e_ldst]
    dstv = slot_of[e_ldst].astype(np.float32)
    wv = drecip[dst[sel]].astype(np.float32)
    is_lo = e_src < LO_SPLIT
    halves = {}
    for name, mask, base in (("lo", is_lo, 0), ("hi", ~is_lo, LO_SPLIT)):
        g_lists = []
        for g in range(GROUPS_PER_CORE):
            m = mask & (grp == g)
            g_lists.append(
                ((e_src[m] - base).astype(np.int16), dstv[m], wv[m])
            )
        halves[name] = g_lists
    # node_of: flat slot index -> original local node (or -1)
    node_of = np.full(SLOTS_PER_CORE, -1, np.int64)
    node_of[group_of * 128 + slot_of] = np.arange(NODES_PER_CORE)
    return halves, node_of


def _pad_streams(g_lists, tiles_per_group):
    """Concatenate per-group edge lists padded to tiles_per_group[g]*128.

    Returns idx stream int16, dstv/wv [128, T_total] f32 (column t = tile t)."""
    idx_parts, dstv_parts, wv_parts = [], [], []
    for g, (idx, dv, wv) in enumerate(g_lists):
        cap = int(tiles_per_group[g]) * 128
        pad = cap - idx.shape[0]
        assert pad >= 0
        idx_parts.append(np.concatenate([idx, np.zeros(pad, np.int16)]))
        dstv_parts.append(np.concatenate([dv, np.full(pad, SENTINEL, np.float32)]))
        wv_parts.append(np.concatenate([wv, np.zeros(pad, np.float32)]))
    idx = np.concatenate(idx_parts)
    dstv = np.concatenate(dstv_parts).reshape(-1, 128).T.copy()
    wv = np.concatenate(wv_parts).reshape(-1, 128).T.copy()
    return idx, dstv, wv


def _build_graph(t_lo, t_hi):
    """Build the SPMD Bass graph for the shared (t_lo, t_hi) schedule."""
    _apply_tile_patch()
    nc = bacc.Bacc("TRN2", target_bir_lowering=False, debug=False)
    n_hi_rows = N_NODES - LO_SPLIT
    T_LO = int(np.sum(t_lo))
    T_HI = int(np.sum(t_hi))
    T_MAX = int(max(np.max(t_lo + t_hi), 1))

    feat_lo = nc.declare_dram_parameter(
        "feat_lo", [LO_SPLIT, D], mybir.dt.float32, isOutput=False
    )
    feat_hi = nc.declare_dram_parameter(
        "feat_hi", [n_hi_rows, D], mybir.dt.float32, isOutput=False
    )
    featT = nc.declare_dram_parameter(
        "featT", [D, SLOTS_PER_CORE], mybir.dt.float32, isOutput=False
    )
    idx_lo = nc.declare_dram_parameter(
        "idx_lo", [128, T_LO * 8], mybir.dt.int16, isOutput=False
    )
    idx_hi = nc.declare_dram_parameter(
        "idx_hi", [128, T_HI * 8], mybir.dt.int16, isOutput=False
    )
    dstv_lo_d = nc.declare_dram_parameter(
        "dstv_lo", [128, T_LO], mybir.dt.float32, isOutput=False
    )
    wv_lo_d = nc.declare_dram_parameter(
        "wv_lo", [128, T_LO], mybir.dt.float32, isOutput=False
    )
    dstv_hi_d = nc.declare_dram_parameter(
        "dstv_hi", [128, T_HI], mybir.dt.float32, isOutput=False
    )
    wv_hi_d = nc.declare_dram_parameter(
        "wv_hi", [128, T_HI], mybir.dt.float32, isOutput=False
    )
    w1t_d = nc.declare_dram_parameter("w1t", [D, D_OUT], mybir.dt.float32, isOutput=False)
    w2t_d = nc.declare_dram_parameter("w2t", [D, D_OUT], mybir.dt.float32, isOutput=False)
    b_d = nc.declare_dram_parameter("bias", [1, D_OUT], mybir.dt.float32, isOutput=False)
    iota_d = nc.declare_dram_parameter(
        "iota", [128, T_MAX * 128], mybir.dt.float32, isOutput=False
    )
    out_d = nc.declare_dram_parameter(
        "out", [SLOTS_PER_CORE, D_OUT], mybir.dt.float32, isOutput=True
    )

    nc.gpsimd.load_library(library_config.mlp)

    chunks = []
    for c0 in range(0, GROUPS_PER_CORE, G_CHUNK):
        chunks.append(list(range(c0, min(c0 + G_CHUNK, GROUPS_PER_CORE))))
    lo_tile_base = np.concatenate([[0], np.cumsum(t_lo)]).astype(int)
    hi_tile_base = np.concatenate([[0], np.cumsum(t_hi)]).astype(int)

    with TileContext(nc) as tc:
        with (
            tc.tile_pool(name="const", bufs=1) as constp,
            tc.tile_pool(name="glo", bufs=3) as glop,
            tc.tile_pool(name="idxp", bufs=3) as idxp,
            tc.tile_pool(name="ghi", bufs=3) as ghip,
            tc.tile_pool(name="stile", bufs=2) as sp,
            tc.tile_pool(name="xt", bufs=3) as xtp,
            tc.tile_pool(name="ft", bufs=3) as ftp,
            tc.tile_pool(name="ostage", bufs=3) as op,
            tc.tile_pool(name="psum_h", bufs=2, space="PSUM") as ph,
            tc.tile_pool(name="psum_o", bufs=2, space="PSUM") as po,
        ):
            def emit_gathers(chunk):
                glo_t0 = int(lo_tile_base[chunk[0]])
                glo_t1 = int(lo_tile_base[chunk[-1] + 1])
                ghi_t0 = int(hi_tile_base[chunk[0]])
                ghi_t1 = int(hi_tile_base[chunk[-1] + 1])
                n_lo_t = glo_t1 - glo_t0
                n_hi_t = ghi_t1 - ghi_t0
                it_lo = idxp.tile([128, n_lo_t * 8], mybir.dt.int16, tag="ilo")
                nc.sync.dma_start(
                    out=it_lo[:], in_=idx_lo[:, glo_t0 * 8 : glo_t1 * 8]
                )
                glo = glop.tile([128, n_lo_t, D], mybir.dt.float32, tag="glo")
                nidx = n_lo_t * 128
                nc.gpsimd.dma_gather(
                    glo[:], feat_lo[:], it_lo[:], nidx, nidx, D,
                    single_packet=False,
                )
                ghi = None
                if n_hi_t > 0:
                    it_hi = idxp.tile([128, n_hi_t * 8], mybir.dt.int16, tag="ihi")
                    nc.sync.dma_start(
                        out=it_hi[:], in_=idx_hi[:, ghi_t0 * 8 : ghi_t1 * 8]
                    )
                    ghi = ghip.tile([128, n_hi_t, D], mybir.dt.float32, tag="ghi")
                    nidx_h = n_hi_t * 128
                    nc.gpsimd.dma_gather(
                        ghi[:], feat_hi[:], it_hi[:], nidx_h, nidx_h, D,
                        single_packet=False,
                    )
                return glo, ghi, glo_t0, ghi_t0

            # chunk 0's idx loads + gathers go first so the Q7 starts
            # immediately; const loads follow and hide under the first gather.
            chunk0_handles = emit_gathers(chunks[0])

            dstv_lo_sb = constp.tile([128, T_LO], mybir.dt.float32)
            nc.scalar.dma_start(out=dstv_lo_sb[:], in_=dstv_lo_d[:])
            wv_lo_sb = constp.tile([128, T_LO], mybir.dt.float32)
            nc.scalar.dma_start(out=wv_lo_sb[:], in_=wv_lo_d[:])
            dstv_hi_sb = constp.tile([128, T_HI], mybir.dt.float32)
            nc.scalar.dma_start(out=dstv_hi_sb[:], in_=dstv_hi_d[:])
            wv_hi_sb = constp.tile([128, T_HI], mybir.dt.float32)
            nc.scalar.dma_start(out=wv_hi_sb[:], in_=wv_hi_d[:])
            iota_sb = constp.tile([128, T_MAX * 128], mybir.dt.float32)
            nc.scalar.dma_start(out=iota_sb[:], in_=iota_d[:])
            w1t_sb = constp.tile([D, D_OUT], mybir.dt.float32)
            nc.scalar.dma_start(out=w1t_sb[:], in_=w1t_d[:])
            w2t_sb = constp.tile([D, D_OUT], mybir.dt.float32)
            nc.scalar.dma_start(out=w2t_sb[:], in_=w2t_d[:])
            b_sb = constp.tile([1, D_OUT], mybir.dt.float32)
            nc.scalar.dma_start(out=b_sb[:], in_=b_d[:])
            ones_sb = constp.tile([1, 128], mybir.dt.float32)
            nc.vector.memset(ones_sb[:], 1.0)

            for ci, chunk in enumerate(chunks):
                if ci == 0:
                    glo, ghi, glo_t0, ghi_t0 = chunk0_handles
                else:
                    glo, ghi, glo_t0, ghi_t0 = emit_gathers(chunk)

                for g in chunk:
                    n_lo = int(t_lo[g])
                    n_hi = int(t_hi[g])
                    n_tot = n_lo + n_hi
                    # batched one-hot build: S[e, (t, n)] =
                    #   (dstv[e, t] == n) * wv[e, t]
                    s_all = sp.tile([128, n_tot * 128], mybir.dt.float32, tag="stile")
                    lo_b = int(lo_tile_base[g])
                    hi_b = int(hi_tile_base[g])
                    nc.vector.tensor_tensor(
                        out=s_all[:, : n_lo * 128],
                        in0=iota_sb[:, : n_lo * 128],
                        in1=dstv_lo_sb[:, lo_b : lo_b + n_lo].to_broadcast(
                            [128, n_lo, 128]
                        ),
                        op=mybir.AluOpType.is_equal,
                    )
                    if n_hi > 0:
                        nc.vector.tensor_tensor(
                            out=s_all[:, n_lo * 128 :],
                            in0=iota_sb[:, : n_hi * 128],
                            in1=dstv_hi_sb[:, hi_b : hi_b + n_hi].to_broadcast(
                                [128, n_hi, 128]
                            ),
                            op=mybir.AluOpType.is_equal,
                        )
                    wvb = sp.tile([128, n_tot * 128], mybir.dt.float32, tag="wvb")
                    nc.vector.tensor_tensor(
                        out=wvb[:, : n_lo * 128],
                        in0=s_all[:, : n_lo * 128],
                        in1=wv_lo_sb[:, lo_b : lo_b + n_lo].to_broadcast(
                            [128, n_lo, 128]
                        ),
                        op=mybir.AluOpType.mult,
                    )
                    if n_hi > 0:
                        nc.vector.tensor_tensor(
                            out=wvb[:, n_lo * 128 :],
                            in0=s_all[:, n_lo * 128 :],
                            in1=wv_hi_sb[:, hi_b : hi_b + n_hi].to_broadcast(
                                [128, n_hi, 128]
                            ),
                            op=mybir.AluOpType.mult,
                        )

                    hT = ph.tile([D, 128], mybir.dt.float32, space="PSUM")
                    for i in range(n_tot):
                        if i < n_lo:
                            msg_ap = glo[:, lo_b + i - glo_t0, :]
                        else:
                            msg_ap = ghi[:, hi_b + (i - n_lo) - ghi_t0, :]
                        nc.tensor.matmul(
                            out=hT[:],
                            lhsT=msg_ap,
                            rhs=wvb[:, i * 128 : (i + 1) * 128],
                            start=(i == 0),
                            stop=(i == n_tot - 1),
                        )
                    xt = xtp.tile([D, 128], mybir.dt.float32, tag="xt")
                    nc.scalar.copy(out=xt[:], in_=hT[:])
                    ft = ftp.tile([D, 128], mybir.dt.float32, tag="ft")
                    nc.scalar.dma_start(
                        out=ft[:], in_=featT[:, g * 128 : (g + 1) * 128]
                    )
                    om = po.tile([128, D_OUT], mybir.dt.float32, space="PSUM")
                    nc.tensor.matmul(
                        out=om[:], lhsT=xt[:], rhs=w1t_sb[:], start=True, stop=False
                    )
                    nc.tensor.matmul(
                        out=om[:], lhsT=ft[:], rhs=w2t_sb[:], start=False, stop=False
                    )
                    nc.tensor.matmul(
                        out=om[:], lhsT=ones_sb[:], rhs=b_sb[:], start=False, stop=True
                    )
                    ost = op.tile([128, D_OUT], mybir.dt.float32, tag="ostage")
                    nc.scalar.copy(out=ost[:], in_=om[:])
                    nc.sync.dma_start(
                        out=out_d[g * 128 : (g + 1) * 128, :], in_=ost[:]
                    )

    nc.finalize()
    return nc


def kernel(feature, src, dst, W, b):
    feature = np.asarray(feature, dtype=np.float32)
    src = np.asarray(src).astype(np.int64)
    dst = np.asarray(dst).astype(np.int64)
    W = np.asarray(W, dtype=np.float32)
    b = np.asarray(b, dtype=np.float32)

    deg = np.bincount(dst, minlength=N_NODES).astype(np.float32)
    drecip = 1.0 / np.maximum(deg, 1.0)

    prepped = [_prep_core(src, dst, drecip, deg, c) for c in range(N_CORES)]

    t_lo = np.zeros(GROUPS_PER_CORE, np.int64)
    t_hi = np.zeros(GROUPS_PER_CORE, np.int64)
    for halves, _ in prepped:
        for g in range(GROUPS_PER_CORE):
            t_lo[g] = max(t_lo[g], (halves["lo"][g][0].shape[0] + 127) // 128)
            t_hi[g] = max(t_hi[g], (halves["hi"][g][0].shape[0] + 127) // 128)
    t_lo = np.maximum(t_lo, 1)  # guarantee a start=True matmul per group

    nc = _build_graph(t_lo, t_hi)

    T_MAX = int(max(np.max(t_lo + t_hi), 1))
    iota = np.tile(np.arange(128, dtype=np.float32), (128, T_MAX))
    w1t = np.ascontiguousarray(W[:, :D].T)
    w2t = np.ascontiguousarray(W[:, D:].T)
    feat_lo = feature[:LO_SPLIT]
    feat_hi = np.ascontiguousarray(feature[LO_SPLIT:])

    in_maps = []
    node_ofs = []
    for c in range(N_CORES):
        halves, node_of = prepped[c]
        node_ofs.append(node_of)
        ilo, dvlo, wvlo = _pad_streams(halves["lo"], t_lo)
        ihi, dvhi, wvhi = _pad_streams(halves["hi"], t_hi)
        base = c * NODES_PER_CORE
        featT_c = np.zeros((D, SLOTS_PER_CORE), np.float32)
        valid = node_of >= 0
        featT_c[:, valid] = feature[base + node_of[valid]].T
        in_maps.append(
            {
                "feat_lo": feat_lo,
                "feat_hi": feat_hi,
                "featT": featT_c,
                "idx_lo": _wrap_idxs(ilo),
                "idx_hi": _wrap_idxs(ihi)
                if ihi.shape[0]
                else np.zeros((128, 0), np.int16),
                "dstv_lo": dvlo,
                "wv_lo": wvlo,
                "dstv_hi": dvhi,
                "wv_hi": wvhi,
                "w1t": w1t,
                "w2t": w2t,
                "bias": b.reshape(1, D_OUT),
                "iota": iota,
            }
        )

    res = run_bass_kernel_spmd(nc, in_maps, list(range(N_CORES)), trace=False)
    out = np.empty((N_NODES, D_OUT), np.float32)
    for c in range(N_CORES):
        rows = np.asarray(res.results[c]["out"])
        node_of = node_ofs[c]
        valid = node_of >= 0
        out[c * NODES_PER_CORE + node_of[valid]] = rows[valid]
    return out



# revision 2
# speedup vs baseline: 1.6707x; 1.6707x over previous
"""GCN layer (gather -> segment-mean -> concat -> linear) on 8 TRN2 NeuronCores.

v2 strategy (dst-sharded, fp16 data path, 4-way-parallel SWDGE gather):
  - 50000 output nodes split across 8 cores (6250 each); per core nodes are
    bin-packed into 49 groups of <=128 balancing degree sums.
  - feature table cast to fp16 host-side; per-edge rows fetched with the
    GPSIMD dma_gather (int16 idx => lo half [0, 32768) + hi half rest).
  - Descriptor generation is the bottleneck engine: dma_gather ucode runs on
    Q7 core pair (2*queue_num, 2*queue_num+1), so gathers are issued
    round-robin on 4 SWDGE queues to generate descriptors 4-way parallel.
  - Segment-sum on TensorE: psum_hT[D, n] += matmul(lhsT=msgs[e, D] fp16,
    rhs=S[e, n] fp16) with S[e, n] = (dstv[e] == n) pure one-hot built by a
    single batched DVE is_equal; the 1/deg mean scaling is applied after
    aggregation (per output row), not per edge.
  - Output: om1[n, dout] = xt.T @ w1t (xt = fp16 copy of psum hT);
    om2[n, dout] = ft.T @ w2t + ones.T @ b; out = om1 * drecip[n] + om2.
"""

import sys

for _p in ("/opt/trn_rl_repo",):
    if _p not in sys.path:
        sys.path.insert(0, _p)

import numpy as np

import concourse.bass as bass
import concourse.mybir as mybir
from concourse import bacc, library_config
from concourse.bass_utils import run_bass_kernel_spmd
from concourse.tile import TileContext
from concourse.vector_clock import ScopedClock

N_NODES = 50000
N_EDGES = 800000
D = 128
D_OUT = 128
N_CORES = 8
NODES_PER_CORE = N_NODES // N_CORES  # 6250
GROUPS_PER_CORE = (NODES_PER_CORE + 127) // 128  # 49
SLOTS_PER_CORE = GROUPS_PER_CORE * 128  # 6272 (padded)
LO_SPLIT = 32768  # int16-index limit for dma_gather
SENTINEL = 1000.0  # dstv value that matches no iota column
G_CHUNK = 4  # groups per dma_gather call
N_QUEUES = 4  # SWDGE queues (Q7 core pairs) used round-robin
F16 = mybir.dt.float16


def _patched_drain_and_barrier(self, tick_clock, wait_clock):
    # The staged walrus build rejects Drain instructions carrying more than
    # one sem wait; split the tail-drain waits onto individual nops.
    probe = self.nc.sync.nop()
    if probe.ins.sync_info is None:
        probe.ins.sync_info = mybir.SyncInfo(on_wait=[], on_update=[])
    wait_clock.add_sem_waits(probe.ins, ScopedClock({None: tick_clock.global_clock}))
    si = probe.ins.sync_info
    waits = list(si.on_wait or [])
    si.on_wait = waits[:1]
    for w in waits[1:]:
        n = self.nc.sync.nop()
        n.ins.sync_info = mybir.SyncInfo(on_wait=[w], on_update=[])
    self.nc.sync.drain()
    self.nc.all_engine_barrier()
    popped = self.nc._tile_sem_poison_stack.pop()
    assert popped is self._sem_poison
    self.nc.clear_and_free_semaphores(list(self.sems.allocated().values()))
    self.nc.all_engine_barrier()


def _apply_tile_patch():
    import concourse.tile as ctile

    ctile.TileContext._drain_and_barrier = _patched_drain_and_barrier


def _wrap_idxs(flat):
    """[N] int16 -> [128, N//16]: position i at [i%16 + 16k, i//16], k=0..7."""
    n = flat.shape[0]
    assert n % 16 == 0
    arr = flat.reshape(n // 16, 16).T
    return np.ascontiguousarray(np.tile(arr, (8, 1)))


def _pack_groups(deg_slice):
    """Greedy balanced bin-packing of 6250 nodes into 49 groups of <=128."""
    n = deg_slice.shape[0]
    order = np.argsort(-deg_slice, kind="stable")
    loads = np.zeros(GROUPS_PER_CORE)
    counts = np.zeros(GROUPS_PER_CORE, np.int64)
    group_of = np.zeros(n, np.int64)
    slot_of = np.zeros(n, np.int64)
    for node in order:
        masked = np.where(counts < 128, loads, np.inf)
        g = int(np.argmin(masked))
        group_of[node] = g
        slot_of[node] = counts[g]
        counts[g] += 1
        loads[g] += deg_slice[node]
    return group_of, slot_of


def _prep_core(src, dst, deg, core):
    """Host-side partitioning for one core: bin-packed groups + per-group
    per-half edge lists (idx, dstv)."""
    lo_node = core * NODES_PER_CORE
    hi_node = lo_node + NODES_PER_CORE
    deg_slice = deg[lo_node:hi_node]
    group_of, slot_of = _pack_groups(deg_slice)

    sel = (dst >= lo_node) & (dst < hi_node)
    e_src = src[sel]
    e_ldst = dst[sel] - lo_node
    grp = group_of[e_ldst]
    dstv = slot_of[e_ldst].astype(np.float32)
    is_lo = e_src < LO_SPLIT
    halves = {}
    for name, mask, base in (("lo", is_lo, 0), ("hi", ~is_lo, LO_SPLIT)):
        g_lists = []
        for g in range(GROUPS_PER_CORE):
            m = mask & (grp == g)
            g_lists.append(((e_src[m] - base).astype(np.int16), dstv[m]))
        halves[name] = g_lists
    node_of = np.full(SLOTS_PER_CORE, -1, np.int64)
    node_of[group_of * 128 + slot_of] = np.arange(NODES_PER_CORE)
    return halves, node_of


def _pad_streams(g_lists, tiles_per_group):
    """Concatenate per-group edge lists padded to tiles_per_group[g]*128."""
    idx_parts, dstv_parts = [], []
    for g, (idx, dv) in enumerate(g_lists):
        cap = int(tiles_per_group[g]) * 128
        pad = cap - idx.shape[0]
        assert pad >= 0
        idx_parts.append(np.concatenate([idx, np.zeros(pad, np.int16)]))
        dstv_parts.append(np.concatenate([dv, np.full(pad, SENTINEL, np.float32)]))
    idx = np.concatenate(idx_parts)
    dstv = np.concatenate(dstv_parts).reshape(-1, 128).T.astype(np.float16).copy()
    return idx, dstv


def _build_graph(t_lo, t_hi):
    """Build the SPMD Bass graph for the shared (t_lo, t_hi) schedule."""
    _apply_tile_patch()
    nc = bacc.Bacc(
        "TRN2",
        target_bir_lowering=False,
        debug=False,
        num_swdge_queues=N_QUEUES,
    )
    n_hi_rows = N_NODES - LO_SPLIT
    T_LO = int(np.sum(t_lo))
    T_HI = int(np.sum(t_hi))
    T_MAX = int(max(np.max(t_lo + t_hi), 1))

    feat_lo = nc.declare_dram_parameter("feat_lo", [LO_SPLIT, D], F16, isOutput=False)
    feat_hi = nc.declare_dram_parameter("feat_hi", [n_hi_rows, D], F16, isOutput=False)
    featT = nc.declare_dram_parameter(
        "featT", [D, SLOTS_PER_CORE], F16, isOutput=False
    )
    idx_lo = nc.declare_dram_parameter(
        "idx_lo", [128, T_LO * 8], mybir.dt.int16, isOutput=False
    )
    idx_hi = nc.declare_dram_parameter(
        "idx_hi", [128, T_HI * 8], mybir.dt.int16, isOutput=False
    )
    dstv_lo_d = nc.declare_dram_parameter("dstv_lo", [128, T_LO], F16, isOutput=False)
    dstv_hi_d = nc.declare_dram_parameter("dstv_hi", [128, T_HI], F16, isOutput=False)
    w1t_d = nc.declare_dram_parameter("w1t", [D, D_OUT], F16, isOutput=False)
    w2t_d = nc.declare_dram_parameter("w2t", [D, D_OUT], F16, isOutput=False)
    b_d = nc.declare_dram_parameter("bias", [1, D_OUT], F16, isOutput=False)
    drec_d = nc.declare_dram_parameter(
        "drec", [128, GROUPS_PER_CORE], mybir.dt.float32, isOutput=False
    )
    iota_d = nc.declare_dram_parameter("iota", [128, T_MAX * 128], F16, isOutput=False)
    out_d = nc.declare_dram_parameter(
        "out", [SLOTS_PER_CORE, D_OUT], mybir.dt.float32, isOutput=True
    )

    nc.gpsimd.load_library(library_config.mlp)

    chunks = []
    for c0 in range(0, GROUPS_PER_CORE, G_CHUNK):
        chunks.append(list(range(c0, min(c0 + G_CHUNK, GROUPS_PER_CORE))))
    lo_tile_base = np.concatenate([[0], np.cumsum(t_lo)]).astype(int)
    hi_tile_base = np.concatenate([[0], np.cumsum(t_hi)]).astype(int)

    qctr = [0]

    with TileContext(nc) as tc:
        with (
            tc.tile_pool(name="const", bufs=1) as constp,
            tc.tile_pool(name="glo", bufs=8) as glop,
            tc.tile_pool(name="idxp", bufs=16) as idxp,
            tc.tile_pool(name="ghi", bufs=8) as ghip,
            tc.tile_pool(name="stile", bufs=3) as sp,
            tc.tile_pool(name="xt", bufs=3) as xtp,
            tc.tile_pool(name="ft", bufs=3) as ftp,
            tc.tile_pool(name="ostage", bufs=3) as op,
            tc.tile_pool(name="psum_h", bufs=2, space="PSUM") as ph,
            tc.tile_pool(name="psum_o1", bufs=2, space="PSUM") as po1,
            tc.tile_pool(name="psum_o2", bufs=2, space="PSUM") as po2,
        ):
            def _emit_piece(half_d, idx_d_t, t0, t1, tag, queue):
                """One dma_gather over tile range [t0, t1) of a half."""
                n_t = t1 - t0
                if n_t <= 0:
                    return None
                it = idxp.tile([128, n_t * 8], mybir.dt.int16, tag="i" + tag)
                nc.sync.dma_start(out=it[:], in_=idx_d_t[:, t0 * 8 : t1 * 8])
                if tag == "lo":
                    g = glop.tile([128, n_t, D], F16, tag="glo")
                else:
                    g = ghip.tile([128, n_t, D], F16, tag="ghi")
                nidx = n_t * 128
                nc.gpsimd.dma_gather(
                    g[:], half_d[:], it[:], nidx, nidx, D,
                    single_packet=False, queue_num=queue,
                )
                return g

            def emit_gathers(chunk):
                """Split each chunk's lo/hi idx streams in two and spread the
                four gathers across the four SWDGE queues (rotating so every
                queue sees the same lo/hi mix across chunks)."""
                L0 = int(lo_tile_base[chunk[0]])
                L1 = int(lo_tile_base[chunk[-1] + 1])
                H0 = int(hi_tile_base[chunk[0]])
                H1 = int(hi_tile_base[chunk[-1] + 1])
                ML = (L0 + L1 + 1) // 2
                MH = (H0 + H1 + 1) // 2
                qr = qctr[0]
                qctr[0] = (qctr[0] + 2) % N_QUEUES
                glo_a = _emit_piece(feat_lo, idx_lo, L0, ML, "lo", qr % 4)
                glo_b = _emit_piece(feat_lo, idx_lo, ML, L1, "lo", (qr + 1) % 4)
                ghi_a = _emit_piece(feat_hi, idx_hi, H0, MH, "hi", (qr + 2) % 4)
                ghi_b = _emit_piece(feat_hi, idx_hi, MH, H1, "hi", (qr + 3) % 4)
                return (glo_a, glo_b, ghi_a, ghi_b, L0, ML, H0, MH)

            # first chunks' gathers go first so the Q7s start immediately;
            # const loads follow and hide under the first gathers.
            pre = [emit_gathers(chunks[i]) for i in range(min(4, len(chunks)))]

            dstv_lo_sb = constp.tile([128, T_LO], F16)
            nc.scalar.dma_start(out=dstv_lo_sb[:], in_=dstv_lo_d[:])
            dstv_hi_sb = constp.tile([128, T_HI], F16)
            nc.scalar.dma_start(out=dstv_hi_sb[:], in_=dstv_hi_d[:])
            iota_sb = constp.tile([128, T_MAX * 128], F16)
            nc.scalar.dma_start(out=iota_sb[:], in_=iota_d[:])
            w1t_sb = constp.tile([D, D_OUT], F16)
            nc.scalar.dma_start(out=w1t_sb[:], in_=w1t_d[:])
            w2t_sb = constp.tile([D, D_OUT], F16)
            nc.scalar.dma_start(out=w2t_sb[:], in_=w2t_d[:])
            b_sb = constp.tile([1, D_OUT], F16)
            nc.scalar.dma_start(out=b_sb[:], in_=b_d[:])
            drec_sb = constp.tile([128, GROUPS_PER_CORE], mybir.dt.float32)
            nc.scalar.dma_start(out=drec_sb[:], in_=drec_d[:])
            ones_sb = constp.tile([1, 128], F16)
            nc.vector.memset(ones_sb[:], 1.0)

            for ci, chunk in enumerate(chunks):
                if ci < len(pre):
                    glo_a, glo_b, ghi_a, ghi_b, L0, ML, H0, MH = pre[ci]
                else:
                    glo_a, glo_b, ghi_a, ghi_b, L0, ML, H0, MH = emit_gathers(chunk)

                for g in chunk:
                    n_lo = int(t_lo[g])
                    n_hi = int(t_hi[g])
                    n_tot = n_lo + n_hi
                    # batched one-hot build: S[e, (t, n)] = (dstv[e, t] == n)
                    s_all = sp.tile([128, n_tot * 128], F16, tag="stile")
                    lo_b = int(lo_tile_base[g])
                    hi_b = int(hi_tile_base[g])
                    nc.vector.tensor_tensor(
                        out=s_all[:, : n_lo * 128],
                        in0=dstv_lo_sb[:, lo_b : lo_b + n_lo].to_broadcast(
                            [128, n_lo, 128]
                        ),
                        in1=iota_sb[:, : n_lo * 128],
                        op=mybir.AluOpType.is_equal,
                    )
                    if n_hi > 0:
                        nc.vector.tensor_tensor(
                            out=s_all[:, n_lo * 128 :],
                            in0=dstv_hi_sb[:, hi_b : hi_b + n_hi].to_broadcast(
                                [128, n_hi, 128]
                            ),
                            in1=iota_sb[:, : n_hi * 128],
                            op=mybir.AluOpType.is_equal,
                        )

                    hT = ph.tile([D, 128], mybir.dt.float32, space="PSUM")
                    for i in range(n_tot):
                        if i < n_lo:
                            t = lo_b + i
                            msg_ap = (
                                glo_a[:, t - L0, :] if t < ML
                                else glo_b[:, t - ML, :]
                            )
                        else:
                            t = hi_b + (i - n_lo)
                            msg_ap = (
                                ghi_a[:, t - H0, :] if t < MH
                                else ghi_b[:, t - MH, :]
                            )
                        nc.tensor.matmul(
                            out=hT[:],
                            lhsT=msg_ap,
                            rhs=s_all[:, i * 128 : (i + 1) * 128],
                            start=(i == 0),
                            stop=(i == n_tot - 1),
                        )
                    xt = xtp.tile([D, 128], F16, tag="xt")
                    nc.scalar.copy(out=xt[:], in_=hT[:])
                    ft = ftp.tile([D, 128], F16, tag="ft")
                    nc.scalar.dma_start(
                        out=ft[:], in_=featT[:, g * 128 : (g + 1) * 128]
                    )
                    om1 = po1.tile([128, D_OUT], mybir.dt.float32, space="PSUM")
                    nc.tensor.matmul(
                        out=om1[:], lhsT=xt[:], rhs=w1t_sb[:], start=True, stop=True
                    )
                    om2 = po2.tile([128, D_OUT], mybir.dt.float32, space="PSUM")
                    nc.tensor.matmul(
                        out=om2[:], lhsT=ft[:], rhs=w2t_sb[:], start=True, stop=False
                    )
                    nc.tensor.matmul(
                        out=om2[:], lhsT=ones_sb[:], rhs=b_sb[:], start=False,
                        stop=True,
                    )
                    ost = op.tile([128, D_OUT], mybir.dt.float32, tag="ostage")
                    nc.vector.tensor_tensor(
                        out=ost[:],
                        in0=om1[:],
                        in1=drec_sb[:, g : g + 1].to_broadcast([128, 1, 128]),
                        op=mybir.AluOpType.mult,
                    )
                    nc.vector.tensor_tensor(
                        out=ost[:],
                        in0=ost[:],
                        in1=om2[:],
                        op=mybir.AluOpType.add,
                    )
                    nc.sync.dma_start(
                        out=out_d[g * 128 : (g + 1) * 128, :], in_=ost[:]
                    )

    nc.finalize()
    return nc


def kernel(feature, src, dst, W, b):
    feature = np.asarray(feature, dtype=np.float32)
    src = np.asarray(src).astype(np.int64)
    dst = np.asarray(dst).astype(np.int64)
    W = np.asarray(W, dtype=np.float32)
    b = np.asarray(b, dtype=np.float32)

    deg = np.bincount(dst, minlength=N_NODES).astype(np.float32)
    drecip = 1.0 / np.maximum(deg, 1.0)

    prepped = [_prep_core(src, dst, deg, c) for c in range(N_CORES)]

    t_lo = np.zeros(GROUPS_PER_CORE, np.int64)
    t_hi = np.zeros(GROUPS_PER_CORE, np.int64)
    for halves, _ in prepped:
        for g in range(GROUPS_PER_CORE):
            t_lo[g] = max(t_lo[g], (halves["lo"][g][0].shape[0] + 127) // 128)
            t_hi[g] = max(t_hi[g], (halves["hi"][g][0].shape[0] + 127) // 128)
    t_lo = np.maximum(t_lo, 1)  # guarantee a start=True matmul per group

    nc = _build_graph(t_lo, t_hi)

    T_MAX = int(max(np.max(t_lo + t_hi), 1))
    iota = np.tile(np.arange(128, dtype=np.float16), (128, T_MAX))
    feature16 = feature.astype(np.float16)
    w1t = np.ascontiguousarray(W[:, :D].T).astype(np.float16)
    w2t = np.ascontiguousarray(W[:, D:].T).astype(np.float16)
    feat_lo = feature16[:LO_SPLIT]
    feat_hi = np.ascontiguousarray(feature16[LO_SPLIT:])

    in_maps = []
    node_ofs = []
    for c in range(N_CORES):
        halves, node_of = prepped[c]
        node_ofs.append(node_of)
        ilo, dvlo = _pad_streams(halves["lo"], t_lo)
        ihi, dvhi = _pad_streams(halves["hi"], t_hi)
        base = c * NODES_PER_CORE
        valid = node_of >= 0
        featT_c = np.zeros((D, SLOTS_PER_CORE), np.float16)
        featT_c[:, valid] = feature16[base + node_of[valid]].T
        drec_c = np.zeros((128, GROUPS_PER_CORE), np.float32)
        dr_flat = np.zeros(SLOTS_PER_CORE, np.float32)
        dr_flat[valid] = drecip[base + node_of[valid]]
        drec_c[:, :] = dr_flat.reshape(GROUPS_PER_CORE, 128).T
        in_maps.append(
            {
                "feat_lo": feat_lo,
                "feat_hi": feat_hi,
                "featT": featT_c,
                "idx_lo": _wrap_idxs(ilo),
                "idx_hi": _wrap_idxs(ihi)
                if ihi.shape[0]
                else np.zeros((128, 0), np.int16),
                "dstv_lo": dvlo,
                "dstv_hi": dvhi,
                "w1t": w1t,
                "w2t": w2t,
                "bias": b.astype(np.float16).reshape(1, D_OUT),
                "drec": drec_c,
                "iota": iota,
            }
        )

    res = run_bass_kernel_spmd(nc, in_maps, list(range(N_CORES)), trace=False)
    out = np.empty((N_NODES, D_OUT), np.float32)
    for c in range(N_CORES):
        rows = np.asarray(res.results[c]["out"])
        node_of = node_ofs[c]
        valid = node_of >= 0
        out[c * NODES_PER_CORE + node_of[valid]] = rows[valid]
    return out
